# revision 1
# baseline (speedup 1.0000x reference)
"""Multi-head causal attention (B=4, C=2048, E=1024, H=16, D=64) on 8 trn2 cores.

Sharding: core i = (batch b=i//2, head-group g=i%2).  Each core computes its
batch's attention for 8 heads (512 features) and a partial output projection;
the host sums the two partials per batch (W_o split row-wise).

Per-core kernel (all matmuls float32r: full PE rate at N>=256, FP22 operands):
  phase 1: V = x @ Wv_g            -> [tok, 8 heads x (64 feat + ones col)]
           QT/KT per head-pair     -> [128 feat, 2048 tok]   (x.T pre-done on host)
  phase 2: per (head-pair, q-chunk 512, k-block 128):
           S^T = K^T.T @ Q^T       (row-tiled pair, K=64 contraction x 2 heads)
           W^T = exp(S^T / 8)      (one ACT over both heads' psum banks)
           diagonal causal mask    (DVE multiply with host-provided mask)
           hid/rowsum = [V|1].T @ W^T  (M=65 accumulating over k-blocks)
           normalize by 1/rowsum -> hiddenT staged to DRAM
  phase 3: out = hiddenT.T @ Wo_g  (K=512 contraction via 4 chained matmuls)
"""

import numpy as np

import concourse.bass as bass
import concourse.mybir as mybir
import concourse.tile as tile
from concourse.vector_clock import ScopedClock

B, C, E = 4, 2048, 1024
H, D = 16, 64
N_CORES = 8
GF = 512          # features per head-group (8 heads x 64)
HP = 4            # head-pairs per group
QC = 512          # q-chunk width
KB = 128          # k-block width
NQC = C // QC     # 4
NKB = C // KB     # 16
NE = E // 128     # 8 contraction tiles over E
F32 = mybir.dt.float32
F32R = mybir.dt.float32r
BF16 = mybir.dt.bfloat16

_CACHED_NC = None


class PatchedTC(tile.TileContext):
    """This walrus build caps sync waits per instruction (1 for CTRL, ~2 for
    compute ISA structs).  Hoist excess waits onto same-engine NOPs emitted
    just before the instruction (engine streams execute in order, so the
    semantics are identical), and split the end-of-kernel drain's waits
    across single-wait drain instructions."""

    WAIT_CAP = 1

    def _commit_instruction(self, inst, lazy_reg_writes=True):
        si = getattr(inst, "sync_info", None)
        if (
            si is not None
            and len(si.on_wait) > self.WAIT_CAP
            and getattr(inst, "engine", mybir.EngineType.Unassigned)
            != mybir.EngineType.Unassigned
        ):
            waits = list(si.on_wait)
            keep = waits[: self.WAIT_CAP]
            extra = waits[self.WAIT_CAP :]
            si.on_wait[:] = keep
            for w in extra:
                nop = mybir.InstNoOp(
                    name=f"I-nw{self.nc.next_id()}",
                    engine=inst.engine,
                    bass_nofuse=True,
                    sync_info=mybir.SyncInfo(on_wait=[w], on_update=[]),
                )
                super()._commit_instruction(nop, lazy_reg_writes=False)
        return super()._commit_instruction(inst, lazy_reg_writes)

    def _drain_and_barrier(self, tick_clock, wait_clock):
        carrier = self.nc.sync.drain()
        wait_clock.add_sem_waits(
            carrier.ins, ScopedClock({None: tick_clock.global_clock})
        )
        si = carrier.ins.sync_info
        waits = list(si.on_wait) if si is not None else []
        if len(waits) > 1:
            si.on_wait[:] = waits[:1]
            for w in waits[1:]:
                extra = self.nc.sync.drain()
                extra.ins.sync_info = mybir.SyncInfo(on_wait=[w], on_update=[])
        self.nc.all_engine_barrier()
        assert self.sems is not None
        popped = self.nc._tile_sem_poison_stack.pop()
        assert popped is self._sem_poison
        self.nc.clear_and_free_semaphores(list(self.sems.allocated().values()))
        self.nc.all_engine_barrier()


def build_nc():
    nc = bass.Bass("TRN2", target_bir_lowering=False)
    xT = nc.declare_dram_parameter("xT", [E, C], BF16, isOutput=False)
    Wq = nc.declare_dram_parameter("Wq", [E, GF], BF16, isOutput=False)
    Wk = nc.declare_dram_parameter("Wk", [E, GF], BF16, isOutput=False)
    Wv = nc.declare_dram_parameter("Wv", [E, GF], BF16, isOutput=False)
    Wo = nc.declare_dram_parameter("Wo", [GF, E], BF16, isOutput=False)
    msk = nc.declare_dram_parameter("mask", [128, 4 * QC], mybir.dt.bfloat16, isOutput=False)
    out = nc.declare_dram_parameter("out", [C, E], F32, isOutput=True)

    xT_t = xT.ap().rearrange("(po pi) f -> pi po f", pi=128)    # [128, 8, C]
    Wq_t = Wq.ap().rearrange("(po pi) f -> pi po f", pi=128)    # [128, 8, GF]
    Wk_t = Wk.ap().rearrange("(po pi) f -> pi po f", pi=128)
    Wv_t = Wv.ap().rearrange("(po pi) f -> pi po f", pi=128)
    Wo_t = Wo.ap().rearrange("(po pi) f -> pi po f", pi=128)    # [128, 4, E]

    with PatchedTC(nc) as tc:
        import contextlib

        with contextlib.ExitStack() as ctx:
            consts = ctx.enter_context(tc.tile_pool(name="consts", bufs=1))
            dram = ctx.enter_context(tc.tile_pool(name="dram", bufs=1, space="DRAM"))
            ppsum = ctx.enter_context(tc.tile_pool(name="ppsum", bufs=2, space="PSUM"))

            mask_sb = consts.tile([128, 4 * QC], mybir.dt.bfloat16)
            nc.sync.dma_start(mask_sb[:], msk.ap())

            xpool = ctx.enter_context(tc.tile_pool(name="xpool", bufs=1))
            vpool = ctx.enter_context(tc.tile_pool(name="vpool", bufs=1))

            xT_sb = xpool.tile([128, NE, C], BF16)
            for e in range(NE):
                nc.sync.dma_start(xT_sb[:, e, :], xT_t[:, e, :])

            # ---- phase 1a: V for all 8 heads, ones column appended per head
            with tc.tile_pool(name="wvpool", bufs=1) as wvpool:
                wv_sb = wvpool.tile([128, NE, GF], BF16)
                nc.sync.dma_start(wv_sb[:], Wv_t[:])
                v_sb = vpool.tile([128, NKB, 2 * GF], BF16)  # [tok, kb, h*(64V|64ones)]
                nc.any.memset(v_sb[:], 1.0)
                for t in range(NKB):
                    pv = ppsum.tile([128, GF], F32, tag="ppsum")
                    for e in range(NE):
                        nc.tensor.matmul(
                            pv[:],
                            lhsT=xT_sb[:, e, t * 128 : (t + 1) * 128],
                            rhs=wv_sb[:, e, :],
                            start=(e == 0),
                            stop=(e == NE - 1),
                        )
                    dst = v_sb[:, t, :].rearrange("p (h u) -> p h u", u=128)[:, :, 0:64]
                    nc.vector.tensor_copy(dst, pv[:].rearrange("p (h u) -> p h u", u=64))

            # ---- phases 1b + 2: per head-pair projections + attention
            qkpool = ctx.enter_context(tc.tile_pool(name="qkpool", bufs=2))
            wpool = ctx.enter_context(tc.tile_pool(name="wpool", bufs=1))
            stpool = ctx.enter_context(tc.tile_pool(name="stpsum", bufs=2, space="PSUM"))
            hidpool = ctx.enter_context(tc.tile_pool(name="hidpsum", bufs=1, space="PSUM"))
            wtpool = ctx.enter_context(tc.tile_pool(name="wtpool", bufs=2))
            napool = ctx.enter_context(tc.tile_pool(name="napool", bufs=2))
            hidT_dram = dram.tile([HP, 128, C], BF16)

            for hp in range(HP):
                wq_sb = wpool.tile([128, NE, 128], BF16, tag="wq")
                wk_sb = wpool.tile([128, NE, 128], BF16, tag="wk")
                nc.sync.dma_start(wq_sb[:], Wq_t[:, :, hp * 128 : (hp + 1) * 128])
                nc.sync.dma_start(wk_sb[:], Wk_t[:, :, hp * 128 : (hp + 1) * 128])
                # fp32r here: bf16 row-tiled matmul pairs crash the exec unit
                # (NRT_EXEC_UNIT_UNRECOVERABLE); fp32r pairs are stable and the
                # 2 cyc/row fp32r rate over a concurrent pair matches unpaired
                # bf16 anyway.
                qt = qkpool.tile([128, C], F32R, tag="qt")
                kt = qkpool.tile([128, C], F32R, tag="kt")
                for n in range(NQC):
                    pq = ppsum.tile([128, QC], F32, tag="ppsum")
                    for e in range(NE):
                        nc.tensor.matmul(
                            pq[:],
                            lhsT=wq_sb[:, e, :],
                            rhs=xT_sb[:, e, n * QC : (n + 1) * QC],
                            start=(e == 0),
                            stop=(e == NE - 1),
                        )
                    nc.vector.tensor_copy(qt[:, n * QC : (n + 1) * QC], pq[:])
                    pk = ppsum.tile([128, QC], F32, tag="ppsum")
                    for e in range(NE):
                        nc.tensor.matmul(
                            pk[:],
                            lhsT=wk_sb[:, e, :],
                            rhs=xT_sb[:, e, n * QC : (n + 1) * QC],
                            start=(e == 0),
                            stop=(e == NE - 1),
                        )
                    nc.vector.tensor_copy(kt[:, n * QC : (n + 1) * QC], pk[:])

                for qc in range(NQC):
                    nkb = 4 * qc + 4
                    hidA = hidpool.tile([128, QC], F32, tag="hidA")
                    hidB = hidpool.tile([128, QC], F32, tag="hidB")
                    for kb in range(nkb):
                        st = stpool.tile([128, 2 * QC], F32, tag="st")
                        nc.tensor.matmul(
                            st[:, 0:QC],
                            lhsT=kt[0:64, kb * KB : (kb + 1) * KB],
                            rhs=qt[0:64, qc * QC : (qc + 1) * QC],
                            start=True,
                            stop=True,
                        )
                        nc.tensor.matmul(
                            st[:, QC : 2 * QC],
                            lhsT=kt[64:128, kb * KB : (kb + 1) * KB],
                            rhs=qt[64:128, qc * QC : (qc + 1) * QC],
                            start=True,
                            stop=True,
                        )
                        wt = wtpool.tile([128, 2 * QC], BF16, tag="wt")
                        nc.scalar.activation(
                            wt[:], st[:], mybir.ActivationFunctionType.Exp, scale=0.125
                        )
                        dr = kb - (nkb - 4)
                        if dr >= 0:
                            nc.vector.tensor_tensor(
                                wt[:].rearrange("p (a b) -> p a b", a=2),
                                wt[:].rearrange("p (a b) -> p a b", a=2),
                                mask_sb[:, None, dr * QC : (dr + 1) * QC].to_broadcast(
                                    (128, 2, QC)
                                ),
                                mybir.AluOpType.mult,
                            )
                        # hidden rows 0:64; rowsum replicated on rows 64:128
                        # (ones columns embedded in v_sb)
                        nc.tensor.matmul(
                            hidA[:],
                            lhsT=v_sb[:, kb, 2 * hp * 128 : (2 * hp + 1) * 128],
                            rhs=wt[:, 0:QC],
                            start=(kb == 0),
                            stop=(kb == nkb - 1),
                        )
                        nc.tensor.matmul(
                            hidB[:],
                            lhsT=v_sb[:, kb, (2 * hp + 1) * 128 : (2 * hp + 2) * 128],
                            rhs=wt[:, QC : 2 * QC],
                            start=(kb == 0),
                            stop=(kb == nkb - 1),
                        )
                    # 1/rowsum via exp(-ln(rs)) on ACT: DVE's bit-exact
                    # reciprocal is ~6 cycles/elem and custom DVE ops don't
                    # compile on this toolchain; ln/exp share one table set.
                    lnA = napool.tile([64, QC], F32, tag="ln")
                    lnB = napool.tile([64, QC], F32, tag="ln")
                    recA = napool.tile([64, QC], F32, tag="rec")
                    recB = napool.tile([64, QC], F32, tag="rec")
                    nc.scalar.activation(
                        lnA[:], hidA[64:128, :], mybir.ActivationFunctionType.Ln
                    )
                    nc.scalar.activation(
                        lnB[:], hidB[64:128, :], mybir.ActivationFunctionType.Ln
                    )
                    nc.scalar.activation(
                        recA[:], lnA[:], mybir.ActivationFunctionType.Exp, scale=-1.0
                    )
                    nc.scalar.activation(
                        recB[:], lnB[:], mybir.ActivationFunctionType.Exp, scale=-1.0
                    )
                    stage = napool.tile([128, QC], BF16, tag="stage")
                    nc.vector.tensor_tensor(
                        stage[0:64, :], hidA[0:64, :], recA[:], mybir.AluOpType.mult
                    )
                    nc.vector.tensor_tensor(
                        stage[64:128, :], hidB[0:64, :], recB[:], mybir.AluOpType.mult
                    )
                    nc.sync.dma_start(
                        hidT_dram[hp, :, qc * QC : (qc + 1) * QC], stage[:]
                    )

            # ---- phase 3: out projection, contracting all 512 group features
            with tc.tile_pool(name="opool", bufs=1) as opool, tc.tile_pool(
                name="ostage", bufs=3
            ) as ostage:
                wo_sb = opool.tile([128, HP, E], BF16)
                nc.sync.dma_start(wo_sb[:], Wo_t[:])
                hf = opool.tile([128, HP, C], BF16)
                for f in range(HP):
                    nc.sync.dma_start(hf[:, f, :], hidT_dram[f, :, :])
                for qb in range(C // 128):
                    for ec in range(E // QC):
                        po = ppsum.tile([128, QC], F32, tag="ppsum")
                        for f in range(HP):
                            nc.tensor.matmul(
                                po[:],
                                lhsT=hf[:, f, qb * 128 : (qb + 1) * 128],
                                rhs=wo_sb[:, f, ec * QC : (ec + 1) * QC],
                                start=(f == 0),
                                stop=(f == HP - 1),
                            )
                        so = ostage.tile([128, QC], F32, tag="so")
                        nc.vector.tensor_copy(so[:], po[:])
                        nc.sync.dma_start(
                            out.ap()[qb * 128 : (qb + 1) * 128, ec * QC : (ec + 1) * QC],
                            so[:],
                        )
    return nc


def _make_mask():
    import ml_dtypes

    m = np.zeros((128, 4, QC), dtype=np.float32)
    for rr in range(4):
        kk = np.arange(128)[:, None]
        qq = np.arange(QC)[None, :]
        m[:, rr, :] = (128 * rr + kk <= qq).astype(np.float32)
    return np.ascontiguousarray(m.reshape(128, 4 * QC)).astype(ml_dtypes.bfloat16)


def make_in_maps(x, W_q, W_k, W_v, W_o):
    import ml_dtypes

    bf16 = ml_dtypes.bfloat16
    mask = _make_mask()
    in_maps = []
    for i in range(N_CORES):
        b, g = i // 2, i % 2
        in_maps.append(
            {
                "xT": np.ascontiguousarray(np.asarray(x)[b].T).astype(bf16),
                "Wq": np.ascontiguousarray(
                    np.asarray(W_q)[:, g * GF : (g + 1) * GF]
                ).astype(bf16),
                "Wk": np.ascontiguousarray(
                    np.asarray(W_k)[:, g * GF : (g + 1) * GF]
                ).astype(bf16),
                "Wv": np.ascontiguousarray(
                    np.asarray(W_v)[:, g * GF : (g + 1) * GF]
                ).astype(bf16),
                "Wo": np.ascontiguousarray(
                    np.asarray(W_o)[g * GF : (g + 1) * GF, :]
                ).astype(bf16),
                "mask": mask,
            }
        )
    return in_maps


def kernel(x, W_q, W_k, W_v, W_o):
    global _CACHED_NC
    from concourse.bass_utils import run_bass_kernel_spmd

    if _CACHED_NC is None:
        _CACHED_NC = build_nc()
    nc = _CACHED_NC

    in_maps = make_in_maps(x, W_q, W_k, W_v, W_o)
    res = run_bass_kernel_spmd(nc, in_maps, core_ids=list(range(N_CORES)))
    out = np.empty((B, C, E), dtype=np.float32)
    for b in range(B):
        out[b] = res.results[2 * b]["out"] + res.results[2 * b + 1]["out"]
    return out



# revision 9
# speedup vs baseline: 1.0004x; 1.0004x over previous
"""Multi-head causal attention (B=4, C=2048, E=1024, H=16, D=64) on 8 trn2 cores.

Sharding: core i = (batch b=i//2, head-group g=i%2).  Each core computes its
batch's attention for 8 heads (512 features) and a partial output projection;
the host sums the two partials per batch (W_o split row-wise).

Per-core pipeline (ACT-exp is the pacing engine; tensor work is interleaved
into its gaps):
  - Q/K projections in fp8e4 DoubleRow (x and W_q/W_k host-quantized to fp8;
    W_q/W_k host-scaled by 16 to avoid fp8 subnormals and column-permuted so
    the psum partition layout is the packed [4 heads x 32 feats] x 2 slabs
    needed by the score matmuls).  4 DR matmuls per 512-token psum.
  - Scores S^T per head: fp8 DoubleRow, lhsT/rhs = 32-partition slices of the
    packed K^T/Q^T tiles (contraction 64 = 32 partitions x 2 slabs).
    Diagonal q-chunks are column-trimmed (only q >= k-block start computed).
  - exp on ACT with scale 1/2048 (scores carry the 16x16 weight scaling),
    output bf16 wt; triangular mask multiply on DVE for diagonal blocks only.
  - hid = [V|1s]^T @ wt in bf16, accumulated over k-blocks diag-first so the
    trimmed column ranges have a full-width first (start) and last (stop)
    matmul.  Even head of a pair uses [V|ones] (hid rows 0:64, rowsum rows
    64:128); odd head uses [ones|V] so its normalize multiply can write hidT
    rows 64:128 directly (DVE requires aligned SBUF operands; one PSUM input
    with a shifted SBUF operand is allowed).
  - 1/rowsum via exp(-ln(rs)) on ACT (reciprocal's table doesn't coexist with
    exp's; ln/exp share one set), then DVE mult psum-hid x recip -> hidT SBUF.
  - out = hidT^T @ W_o in bf16, interleaved per 4-qb chunk as soon as the
    previous q-chunk's hidT completes; psum -> f32 staging -> DRAM.
Scheduling: qc-outer sweep ordered so quad0's first two q-chunks run while
quad1's QK projections (and streamed V projections) fill the tensor gaps.
"""

import numpy as np

import concourse.bass as bass
import concourse.mybir as mybir
import concourse.tile as tile
from concourse.vector_clock import ScopedClock

B, C, E = 4, 2048, 1024
H, D = 16, 64
N_CORES = 8
GF = 512          # features per head-group (8 heads x 64)
QC = 512          # q-chunk width
KB = 128          # k-block width
NQC = C // QC     # 4
NKB = C // KB     # 16
NE = E // 128     # 8 contraction tiles over E
F32 = mybir.dt.float32
BF16 = mybir.dt.bfloat16
FP8 = mybir.dt.float8e4
DR = mybir.MatmulPerfMode.DoubleRow
SC = 16.0                   # host scale on W_q/W_k (fp8 subnormal avoidance)
EXPSCALE = 0.125 / (SC * SC)  # 1/sqrt(D) / (16*16)

_CACHED_NC = None


class PatchedTC(tile.TileContext):
    """This walrus build caps sync waits per instruction (1 for CTRL, ~2 for
    compute ISA structs).  Hoist excess waits onto same-engine NOPs emitted
    just before the instruction (engine streams execute in order, so the
    semantics are identical), and split the end-of-kernel drain's waits
    across single-wait drain instructions."""

    WAIT_CAP = 1

    def _commit_instruction(self, inst, lazy_reg_writes=True):
        si = getattr(inst, "sync_info", None)
        if (
            si is not None
            and len(si.on_wait) > self.WAIT_CAP
            and getattr(inst, "engine", mybir.EngineType.Unassigned)
            != mybir.EngineType.Unassigned
        ):
            waits = list(si.on_wait)
            keep = waits[: self.WAIT_CAP]
            extra = waits[self.WAIT_CAP :]
            si.on_wait[:] = keep
            for w in extra:
                nop = mybir.InstNoOp(
                    name=f"I-nw{self.nc.next_id()}",
                    engine=inst.engine,
                    bass_nofuse=True,
                    sync_info=mybir.SyncInfo(on_wait=[w], on_update=[]),
                )
                super()._commit_instruction(nop, lazy_reg_writes=False)
        return super()._commit_instruction(inst, lazy_reg_writes)

    def _drain_and_barrier(self, tick_clock, wait_clock):
        carrier = self.nc.sync.drain()
        wait_clock.add_sem_waits(
            carrier.ins, ScopedClock({None: tick_clock.global_clock})
        )
        si = carrier.ins.sync_info
        waits = list(si.on_wait) if si is not None else []
        if len(waits) > 1:
            si.on_wait[:] = waits[:1]
            for w in waits[1:]:
                extra = self.nc.sync.drain()
                extra.ins.sync_info = mybir.SyncInfo(on_wait=[w], on_update=[])
        self.nc.all_engine_barrier()
        assert self.sems is not None
        popped = self.nc._tile_sem_poison_stack.pop()
        assert popped is self._sem_poison
        self.nc.clear_and_free_semaphores(list(self.sems.allocated().values()))
        self.nc.all_engine_barrier()


def build_nc():
    nc = bass.Bass("TRN2", target_bir_lowering=False)
    xTb = nc.declare_dram_parameter("xTb", [E, C], BF16, isOutput=False)
    xT8 = nc.declare_dram_parameter("xT8", [E, C], FP8, isOutput=False)
    Wq8 = nc.declare_dram_parameter("Wq8", [E, GF], FP8, isOutput=False)
    Wk8 = nc.declare_dram_parameter("Wk8", [E, GF], FP8, isOutput=False)
    Wv = nc.declare_dram_parameter("Wv", [E, GF], BF16, isOutput=False)
    Wo = nc.declare_dram_parameter("Wo", [GF, E], BF16, isOutput=False)
    # triangular strip mask (k<=q within a 128x128 diagonal sub-block)
    mtri = nc.declare_dram_parameter("mtri", [128, KB], BF16, isOutput=False)
    # wide masks for qc=0 (zeros below the strip + triangle), width 128*(dr+1)
    mwide = nc.declare_dram_parameter("mwide", [128, 4 * QC], BF16, isOutput=False)
    out = nc.declare_dram_parameter("out", [C, E], F32, isOutput=True)

    xTb_t = xTb.ap().rearrange("(po pi) f -> pi po f", pi=128)  # [128, 8, C]
    xT8_t = xT8.ap().rearrange("(po pi) f -> pi po f", pi=128)
    Wq8_t = Wq8.ap().rearrange("(po pi) f -> pi po f", pi=128)  # [128, 8, 512]
    Wk8_t = Wk8.ap().rearrange("(po pi) f -> pi po f", pi=128)
    Wv_t = Wv.ap().rearrange("(po pi) f -> pi po f", pi=128)
    Wo_t = Wo.ap().rearrange("(po pi) f -> pi po f", pi=128)    # [128, 4, E]

    with PatchedTC(nc) as tc:
        import contextlib

        with contextlib.ExitStack() as ctx:
            consts = ctx.enter_context(tc.tile_pool(name="consts", bufs=1))
            ppsum = ctx.enter_context(tc.tile_pool(name="ppsum", bufs=2, space="PSUM"))
            xpool = ctx.enter_context(tc.tile_pool(name="xpool", bufs=1))
            vpool = ctx.enter_context(tc.tile_pool(name="vpool", bufs=1))
            qkpool = ctx.enter_context(tc.tile_pool(name="qkpool", bufs=1))
            stpool = ctx.enter_context(tc.tile_pool(name="stpsum", bufs=2, space="PSUM"))
            hidpool = ctx.enter_context(tc.tile_pool(name="hidpsum", bufs=1, space="PSUM"))
            wtpool = ctx.enter_context(tc.tile_pool(name="wtpool", bufs=2))
            napool = ctx.enter_context(tc.tile_pool(name="napool", bufs=2))
            hfpool = ctx.enter_context(tc.tile_pool(name="hfpool", bufs=1))
            ostage = ctx.enter_context(tc.tile_pool(name="ostage", bufs=3))

            mtri_sb = consts.tile([128, KB], BF16)
            mwide_sb = consts.tile([128, 4, QC], BF16)
            nc.sync.dma_start(mtri_sb[:], mtri.ap())
            nc.sync.dma_start(
                mwide_sb[:], mwide.ap().rearrange("p (r q) -> p r q", q=QC)
            )

            # DMA issue order tuned for startup: wv + first x_bf16 chunks
            # (V projections), then fp8 weights + x_fp8 (QK projections),
            # then the rest of x_bf16, then Wo (needed last).
            wv_sb = consts.tile([128, NE, GF], BF16)
            wq_sb = consts.tile([128, NE, GF], FP8)
            wk_sb = consts.tile([128, NE, GF], FP8)
            wo_sb = consts.tile([128, 4, E], BF16)
            x8_sb = xpool.tile([128, NE, C], FP8)
            xb_sb = xpool.tile([128, NE, C], BF16)
            nc.sync.dma_start(wv_sb[:], Wv_t[:])
            for t in range(4):
                nc.sync.dma_start(
                    xb_sb[:, :, t * KB : (t + 1) * KB],
                    xTb_t[:, :, t * KB : (t + 1) * KB],
                )
            nc.sync.dma_start(wq_sb[:], Wq8_t[:])
            nc.sync.dma_start(wk_sb[:], Wk8_t[:])
            for e in range(NE):
                nc.sync.dma_start(x8_sb[:, e, :], xT8_t[:, e, :])
            for t in range(4, NKB):
                nc.sync.dma_start(
                    xb_sb[:, :, t * KB : (t + 1) * KB],
                    xTb_t[:, :, t * KB : (t + 1) * KB],
                )
            nc.sync.dma_start(wo_sb[:], Wo_t[:])

            # v_sb: per k-block tile, 8 heads x 128 cols.  Even head of a
            # pair: [V(64) | ones(64)]; odd head: [ones | V].
            v_sb = vpool.tile([128, NKB, 2 * GF], BF16)
            nc.any.memset(v_sb[:], 1.0)

            # packed Q^T/K^T per head-pair: [2 heads x 32 feats, 2 slabs, C]
            # (matmul base partitions are limited to 0/32/64, so four heads
            # cannot share one 128-partition tile)
            qtA = qkpool.tile([64, 2, C], FP8, tag="qtA")
            qtB = qkpool.tile([64, 2, C], FP8, tag="qtB")
            qtC = qkpool.tile([64, 2, C], FP8, tag="qtC")
            qtD = qkpool.tile([64, 2, C], FP8, tag="qtD")
            ktA = qkpool.tile([64, 2, C], FP8, tag="ktA")
            ktB = qkpool.tile([64, 2, C], FP8, tag="ktB")
            ktC = qkpool.tile([64, 2, C], FP8, tag="ktC")
            ktD = qkpool.tile([64, 2, C], FP8, tag="ktD")
            qt = [qtA, qtB, qtC, qtD]
            kt = [ktA, ktB, ktC, ktD]

            hf = hfpool.tile([128, 4, C], BF16)  # hidT: 4 head-pairs x 64+64

            def emit_v(t):
                pv = ppsum.tile([128, GF], F32, tag="ppsum")
                for e in range(NE):
                    nc.tensor.matmul(
                        pv[:],
                        lhsT=xb_sb[:, e, t * KB : (t + 1) * KB],
                        rhs=wv_sb[:, e, :],
                        start=(e == 0),
                        stop=(e == NE - 1),
                    )
                src = pv[:].rearrange("p (h u) -> p h u", u=64)
                dst = v_sb[:, t, :].rearrange("p (h u) -> p h u", u=128)
                nc.vector.tensor_copy(dst[:, 0:8:2, 0:64], src[:, 0:8:2, :])
                nc.vector.tensor_copy(dst[:, 1:8:2, 64:128], src[:, 1:8:2, :])

            def emit_qk(quad, which, slab, n):
                # one 512-token fp8-DoubleRow psum chain producing the packed
                # [4 heads x 32 feats] partitions of slab `slab` for `quad`
                w_sb = wq_sb if which == 0 else wk_sb
                dsts = qt if which == 0 else kt
                col0 = (2 * quad + slab) * 128
                pq = ppsum.tile([128, QC], F32, tag="ppsum")
                for i in range(4):
                    nc.tensor.matmul(
                        pq[:],
                        lhsT=w_sb[:, 2 * i : 2 * i + 2, col0 : col0 + 128],
                        rhs=x8_sb[:, 2 * i : 2 * i + 2, n * QC : (n + 1) * QC],
                        start=(i == 0),
                        stop=(i == 3),
                        perf_mode=DR,
                    )
                nc.vector.tensor_copy(
                    dsts[2 * quad][:, slab, n * QC : (n + 1) * QC], pq[0:64, :]
                )
                nc.vector.tensor_copy(
                    dsts[2 * quad + 1][:, slab, n * QC : (n + 1) * QC],
                    pq[64:128, :],
                )

            def emit_o(qb):
                for ec in range(E // QC):
                    po = ppsum.tile([128, QC], F32, tag="ppsum")
                    for f in range(4):
                        nc.tensor.matmul(
                            po[:],
                            lhsT=hf[:, f, qb * KB : (qb + 1) * KB],
                            rhs=wo_sb[:, f, ec * QC : (ec + 1) * QC],
                            start=(f == 0),
                            stop=(f == 3),
                        )
                    so = ostage.tile([128, QC], F32, tag="so")
                    nc.vector.tensor_copy(so[:], po[:])
                    nc.sync.dma_start(
                        out.ap()[qb * KB : (qb + 1) * KB, ec * QC : (ec + 1) * QC],
                        so[:],
                    )

            # ---- filler work queue --------------------------------------
            # entries: (ready_step, deadline_step, emit_fn).  A filler may
            # only be EMITTED at sweep position >= ready (tile-framework deps
            # are tracked in emission order, so e.g. an O-projection emitted
            # before its q-chunk's hidT writes would read unwritten data);
            # it MUST be emitted before position == deadline to avoid stalls.
            sweep = [
                (0, 0), (0, 1), (1, 0), (1, 1),
                (0, 2), (0, 3), (1, 2), (1, 3),
                (2, 0), (2, 1), (2, 2), (2, 3),
                (3, 0), (3, 1), (3, 2), (3, 3),
            ]
            qc_done_after = {0: 5, 1: 7, 2: 11, 3: 15}

            fq = []
            for t in range(4, 8):
                fq.append((0, 2, lambda t=t: emit_v(t)))
            for slab in range(2):
                for n in range(NQC):
                    for which in range(2):
                        fq.append(
                            (0, 4, lambda s=slab, n=n, w=which: emit_qk(1, w, s, n))
                        )
            for t in range(8, 12):
                fq.append((0, 8, lambda t=t: emit_v(t)))
            for t in range(12, 16):
                fq.append((0, 12, lambda t=t: emit_v(t)))
            for qcj in range(3):
                for qb in range(4 * qcj, 4 * qcj + 4):
                    fq.append(
                        (qc_done_after[qcj] + 1, 16, lambda qb=qb: emit_o(qb))
                    )

            # ---- phase A: V(0..3) + quad0 QK projections ----------------
            for t in range(4):
                emit_v(t)
            for slab in range(2):
                for n in range(NQC):
                    for which in range(2):
                        emit_qk(0, which, slab, n)

            # ---- phase B: attention sweep with interleaved fillers ------
            def pop_due(step):
                # force-emit everything whose deadline has arrived
                i = 0
                while i < len(fq):
                    ready, deadline, fn = fq[i]
                    if deadline <= step:
                        assert ready <= step
                        fq.pop(i)
                        fn()
                    else:
                        i += 1

            def pop_one(step):
                for i, (ready, deadline, fn) in enumerate(fq):
                    if ready <= step:
                        fq.pop(i)
                        fn()
                        return

            for step, (qc, hp) in enumerate(sweep):
                # anything whose deadline is this step must be emitted now
                pop_due(step)
                nkb = 4 * qc + 4
                # diag-first order: full-width dr0 opens the accumulation,
                # a full-width non-diag block closes it (qc=0: see below)
                kbs = [nkb - 4 + dr for dr in range(4)] + list(range(nkb - 4))
                hidA = hidpool.tile([128, QC], F32, tag="hidA")
                hidB = hidpool.tile([128, QC], F32, tag="hidB")
                for j, kb in enumerate(kbs):
                    dr = kb - (nkb - 4)
                    diag = dr >= 0
                    off = 128 * dr if (diag and qc > 0) else 0
                    NP = QC - off
                    st = stpool.tile([128, 2, QC], F32, tag="st")
                    for h in range(2):
                        nc.tensor.matmul(
                            st[:, h, off:QC],
                            lhsT=kt[hp][32 * h : 32 * h + 32, :, kb * KB : (kb + 1) * KB],
                            rhs=qt[hp][32 * h : 32 * h + 32, :, qc * QC + off : (qc + 1) * QC],
                            start=True,
                            stop=True,
                            perf_mode=DR,
                        )
                    wt = wtpool.tile([128, 2, QC], BF16, tag="wt")
                    eoff = 0 if qc == 0 else off
                    nc.scalar.activation(
                        wt[:, :, eoff:QC],
                        st[:, :, eoff:QC],
                        mybir.ActivationFunctionType.Exp,
                        scale=EXPSCALE,
                    )
                    if diag:
                        if qc == 0:
                            # zeros below the strip + triangle, width 128(dr+1)
                            wdr = 128 * (dr + 1)
                            nc.vector.tensor_tensor(
                                wt[:, :, 0:wdr],
                                wt[:, :, 0:wdr],
                                mwide_sb[:, None, dr, 0:wdr].to_broadcast(
                                    (128, 2, wdr)
                                ),
                                mybir.AluOpType.mult,
                            )
                        else:
                            nc.vector.tensor_tensor(
                                wt[:, :, off : off + KB],
                                wt[:, :, off : off + KB],
                                mtri_sb[:, None, :].to_broadcast((128, 2, KB)),
                                mybir.AluOpType.mult,
                            )
                    first, last = (j == 0), (j == len(kbs) - 1)
                    nc.tensor.matmul(
                        hidA[:, off:QC],
                        lhsT=v_sb[:, kb, (2 * hp) * KB : (2 * hp + 1) * KB],
                        rhs=wt[:, 0, off:QC],
                        start=first,
                        stop=last,
                    )
                    nc.tensor.matmul(
                        hidB[:, off:QC],
                        lhsT=v_sb[:, kb, (2 * hp + 1) * KB : (2 * hp + 2) * KB],
                        rhs=wt[:, 1, off:QC],
                        start=first,
                        stop=last,
                    )
                    pop_one(step)
                # normalize: 1/rowsum via exp(-ln(rs)) on ACT; hidA rowsum at
                # rows 64:128, hidB ([ones|V]) rowsum at rows 0:64.
                lnA = napool.tile([64, QC], F32, tag="ln")
                recA = napool.tile([64, QC], F32, tag="rec")
                lnB = napool.tile([64, QC], F32, tag="ln")
                recB = napool.tile([64, QC], F32, tag="rec")
                nc.scalar.activation(
                    lnA[:], hidA[64:128, :], mybir.ActivationFunctionType.Ln
                )
                nc.scalar.activation(
                    recA[:], lnA[:], mybir.ActivationFunctionType.Exp, scale=-1.0
                )
                nc.scalar.activation(
                    lnB[:], hidB[0:64, :], mybir.ActivationFunctionType.Ln
                )
                nc.scalar.activation(
                    recB[:], lnB[:], mybir.ActivationFunctionType.Exp, scale=-1.0
                )
                nc.vector.tensor_tensor(
                    hf[0:64, hp, qc * QC : (qc + 1) * QC],
                    hidA[0:64, :],
                    recA[:],
                    mybir.AluOpType.mult,
                )
                nc.vector.tensor_tensor(
                    hf[64:128, hp, qc * QC : (qc + 1) * QC],
                    hidB[64:128, :],
                    recB[:],
                    mybir.AluOpType.mult,
                )

            # drain remaining fillers + final q-chunk's output projection
            while fq:
                fq.pop(0)[2]()
            for qb in range(12, 16):
                emit_o(qb)
    return nc


def _make_masks():
    import ml_dtypes

    kk = np.arange(128)[:, None]
    tri = (kk <= np.arange(KB)[None, :]).astype(np.float32)
    wide = np.zeros((128, 4, QC), dtype=np.float32)
    for dr in range(4):
        qq = np.arange(QC)[None, :]
        wide[:, dr, :] = (128 * dr + kk <= qq).astype(np.float32)
    return (
        tri.astype(ml_dtypes.bfloat16),
        np.ascontiguousarray(wide.reshape(128, 4 * QC)).astype(ml_dtypes.bfloat16),
    )


def _pack_qk_weight(w):
    """Column-permute one group's [E, 512] Q/K weight so each 128-column
    block cs = (quad, slab) holds [h0 f(32s:32s+32) | h1 ... | h3 ...]."""
    wp = w.reshape(E, 8, 2, 32)          # [E, head, slab, 32]
    cols = []
    for quad in range(2):
        for slab in range(2):
            blk = wp[:, 4 * quad : 4 * quad + 4, slab, :]   # [E, 4, 32]
            cols.append(blk.reshape(E, 128))
    return np.concatenate(cols, axis=1)  # [E, 512]


def make_in_maps(x, W_q, W_k, W_v, W_o):
    import ml_dtypes

    bf16 = ml_dtypes.bfloat16
    e4 = ml_dtypes.float8_e4m3fn
    mtri, mwide = _make_masks()
    in_maps = []
    for i in range(N_CORES):
        b, g = i // 2, i % 2
        xT = np.ascontiguousarray(np.asarray(x)[b].T)
        wq = np.asarray(W_q)[:, g * GF : (g + 1) * GF] * SC
        wk = np.asarray(W_k)[:, g * GF : (g + 1) * GF] * SC
        in_maps.append(
            {
                "xTb": xT.astype(bf16),
                "xT8": xT.astype(e4),
                "Wq8": np.ascontiguousarray(_pack_qk_weight(wq)).astype(e4),
                "Wk8": np.ascontiguousarray(_pack_qk_weight(wk)).astype(e4),
                "Wv": np.ascontiguousarray(
                    np.asarray(W_v)[:, g * GF : (g + 1) * GF]
                ).astype(bf16),
                "Wo": np.ascontiguousarray(
                    np.asarray(W_o)[g * GF : (g + 1) * GF, :]
                ).astype(bf16),
                "mtri": mtri,
                "mwide": mwide,
            }
        )
    return in_maps


def kernel(x, W_q, W_k, W_v, W_o):
    global _CACHED_NC
    from concourse.bass_utils import run_bass_kernel_spmd

    if _CACHED_NC is None:
        _CACHED_NC = build_nc()
    nc = _CACHED_NC

    in_maps = make_in_maps(x, W_q, W_k, W_v, W_o)
    res = run_bass_kernel_spmd(nc, in_maps, core_ids=list(range(N_CORES)))
    out = np.empty((B, C, E), dtype=np.float32)
    for b in range(B):
        out[b] = res.results[2 * b]["out"] + res.results[2 * b + 1]["out"]
    return out
